# revision 44
# baseline (speedup 1.0000x reference)
"""MGE velocity kernel for 8 Trainium2 NeuronCores.

out[n] = R_sc[n] * sqrt(vc2_mge[n] + vc2_bh[n]),   R2 = x^2+y^2+z^2

Key observation: with these inputs (m_bh = 8 -> 10^8 BH mass) the black-hole
term dominates the MGE integral everywhere the data lives:
    x(r2) := vc2_mge / vc2_bh <= 5.8e-5  over r2 in [r2_min, r2_max].
Since v = R_sc*sqrt(bh)*sqrt(1+x) and sqrt(1+x) = 1 + x/2 + O(x^2), dropping
the MGE term entirely changes v by at most x_max/2 ~ 2.9e-5 relative — far
below the 2e-2 gate (and below the fp32 noise of the reference itself).
Moreover R_sc*sqrt(bh) = sqrt(G*10^m_bh) * r2^{-1/4}  (scale cancels), so

    v = exp(-0.25*ln(r2) + C),   C = 0.5*(ln G + m_bh*ln 10).

The kernel computes x_max at runtime from the actual inputs (exp-sum of the
exact Q=64 quadrature on a log grid over the data's r2 range) and only takes
this fast path when x_max < 1e-3; otherwise it falls back to the full
128-term quadrature kernel (proven baseline, bit-identical code path).

Fast-path device program (per core, 131072 points as [128, 1024]):
  - inputs shipped fp16 (halves input DMA bytes; quantization adds ~2.5e-4
    rel err, measured 9e-4 total end-to-end) and packed host-side into one
    dram tensor [128, 3072], chunk-major [x|y|z] per chunk -> 1 input DMA
    per chunk. DMA count matters: each HWDGE descriptor-gen is 625ns on a
    single serialized unit + 650ns DGE delay, so input chunks alternate
    SP/HWDGE and GPSIMD/SWDGE paths to overlap generation.
  - per chunk: ONE fp16 2x-mode DVE op squares the whole [x|y|z] block,
    two fp16 DVE adds combine the sub-blocks into r2
  - Ln then Exp(scale=-0.25, bias=C) on ACT over 512-wide halves (wider
    ACT ops amortize the ~400ns/instruction fixed cost; the (512,512)
    split balances "start early" vs "finish the tail fast"), out as fp16
  - C is baked into the program via a Pool memset (cache-keyed on its
    value) -> no cst input DMA at all
  - one pre-placed InstLoadActFuncSet(natural_log_exp_and_others) serves
    Ln+Exp from a single table: avoids the compiler's greedy per-function
    table reloads (1.28us each, 5 of them in the original baseline)

Timeline (TimelineSim): 11.70us vs 151.7us for the 128-term baseline.
The span is latency-dominated: ~1.97us front (SP seq + HWDGE + DGE to
first byte), ~0.9us DMA-sem, DVE chain, ACT Ln/Exp chain, and a fixed
~2.7us output path (HWDGE+DGE+transfer+sem) after the last Exp.
"""

import numpy as np
from numpy.polynomial.legendre import leggauss

N_CORES = 8
H = W = 1024
N = H * W
P = 128
FN = N // N_CORES // P    # 1024 natural free dim per core
# fast-path pipeline chunk widths (sum = FN); tuned via TimelineSim sweep:
# ch0+ch1 = 512 aligns with the first Ln/Exp pair, smaller tail chunk
CWS = (272, 240, 320, 192)
# which input chunks go via GPSIMD/SWDGE instead of SP/HWDGE (overlaps
# descriptor generation for consecutive chunks; Pool's SWDGE gen is slower
# but runs off the critical HWDGE queue)
IN_SWDGE = (False, True, True, False)
# Ln/Exp/out chunk widths (decoupled from CWS; wider ACT ops amortize the
# ~400ns fixed cost per activation instruction)
LN_CWS = (512, 512)
OUT_FP16 = True
IN_FP16 = True
CST_AFTER = 0             # issue cst on SP after this input chunk
ADD_ENG = ("dve", "dve", "dve", "dve")   # engine for each chunk's two adds
G_CONST = 0.004301
SOFT = 0.0
X_TAYLOR_MAX = 1e-3       # max vc2_mge/vc2_bh for the fast path (err <= x/2)

# ---- slow-path (generic) constants: proven baseline kernel ----
QUAD = 8                  # quadrature nodes for the fallback kernel
K = 16                    # MGE components
M = QUAD * K              # exp terms
G_GRP = 32                # point groups per core (fallback layout)
D = 4                     # duplication factor
F = (N // N_CORES) // G_GRP
NI = M // D               # ACT instructions in fallback main loop

_BASS_CACHE = {}
_ACT_COMBINED_SET = None  # resolved lazily: table containing ln+exp+square


def _combined_act_set_id(nc):
    """Index of the activation-function table that serves ln, exp and
    square together (natural_log_exp_and_others on gen3)."""
    global _ACT_COMBINED_SET
    if _ACT_COMBINED_SET is None:
        import concourse.mybir as mybir
        from concourse.hw_specs import get_activation_tables
        AF = mybir.ActivationFunctionType
        need = {AF.Ln, AF.Exp, AF.Square}
        try:
            tables = list(get_activation_tables(nc.m.arch).values())
        except Exception:
            tables = []
        for idx, funcs in enumerate(tables):
            if need.issubset(funcs):
                _ACT_COMBINED_SET = idx
                break
        else:
            _ACT_COMBINED_SET = -1  # no combined table; let bacc insert loads
    return _ACT_COMBINED_SET


def _build_fast(cws=None, in_swdge=None, ln_cws=None, out_fp16=None,
                cst_after=None, c_imm=None, add_eng=None):
    """v = exp(-0.25*ln(x^2+y^2+z^2) + C), chunk-pipelined, fp16 compute.

    DMA structure: input chunk DMAs alternate between SP (HWDGE path) and
    GPSIMD (SWDGE path) so descriptor generation for consecutive chunks
    overlaps instead of serializing on the single HWDGE unit. The tiny cst
    DMA is sandwiched right after in0 on SP. Outputs go via SP/HWDGE.

    Compute is fp16 (inputs shipped fp16, squares/adds fp16 -> DVE runs in
    2x_1p mode); Ln/Exp on ACT are dtype-agnostic, v is written fp32.
    Measured end-to-end error vs the fp32 reference: ~5.7e-4 max.
    """
    cws = CWS if cws is None else cws
    in_swdge = IN_SWDGE if in_swdge is None else in_swdge
    ln_cws = LN_CWS if ln_cws is None else ln_cws
    out_fp16 = OUT_FP16 if out_fp16 is None else out_fp16
    cst_after = CST_AFTER if cst_after is None else cst_after
    add_eng = ADD_ENG if add_eng is None else add_eng
    key = ("fast", cws, in_swdge, ln_cws, out_fp16, cst_after, c_imm, add_eng)
    if key in _BASS_CACHE:
        return _BASS_CACHE[key]
    import concourse.mybir as mybir
    from concourse import bacc
    from concourse.tile import TileContext

    fp32 = mybir.dt.float32
    fp16 = mybir.dt.float16
    AF = mybir.ActivationFunctionType
    OP = mybir.AluOpType
    nch = len(cws)
    offs = np.concatenate([[0], np.cumsum(cws)]).astype(int)
    loffs = np.concatenate([[0], np.cumsum(ln_cws)]).astype(int)
    assert offs[-1] == FN and len(in_swdge) == nch and loffs[-1] == FN
    out_dt = fp16 if out_fp16 else fp32

    nc = bacc.Bacc("TRN2")
    xyz_in = nc.dram_tensor("xyzp", [P, 3 * FN], fp16, kind="ExternalInput")
    # c_imm: bake the exponent constant C into the program as an immediate
    # (cache-keyed); no cst input tensor or DMA at all
    cst_in = None if c_imm is not None else nc.dram_tensor(
        "cst", [P, 8], fp32, kind="ExternalInput")
    out = nc.dram_tensor("out", [P, FN], out_dt, kind="ExternalOutput")

    with TileContext(nc) as tc:
        with tc.tile_pool(name="singles", bufs=1) as sg:
            set_id = _combined_act_set_id(nc)
            if set_id >= 0:
                nc.scalar.add_instruction(mybir.InstLoadActFuncSet(
                    name=nc.get_next_instruction_name(),
                    act_func_set_id=set_id, ins=[], outs=[]))

            xyz_t = sg.tile([P, 3 * FN], fp16)
            if c_imm is not None:
                cst_t = sg.tile([P, 1], fp32)
                nc.gpsimd.memset(cst_t[:], float(c_imm))
            else:
                cst_t = sg.tile([P, 8], fp32)
            sq = sg.tile([P, 3 * FN], fp16)
            t1 = sg.tile([P, FN], fp16)
            r2 = sg.tile([P, FN], fp16)
            lnr2 = sg.tile([P, FN], fp32)
            v = sg.tile([P, FN], out_dt)

            for ch in range(nch):
                s3 = slice(3 * offs[ch], 3 * offs[ch + 1])
                eng = nc.gpsimd if in_swdge[ch] else nc.sync
                eng.dma_start(xyz_t[:, s3], xyz_in[:, s3])
                if ch == cst_after and cst_in is not None:
                    nc.sync.dma_start(cst_t[:], cst_in[:])

            # elementwise at in-chunk granularity; Ln/Exp/out at ln_cws
            # granularity (wider ACT ops amortize ~400ns fixed cost/op).
            # All three squares of a chunk are ONE fp16 2x DVE op over the
            # packed [x|y|z] block; the adds then combine the sub-blocks.
            li = 0
            for ch in range(nch):
                o, cw = 3 * offs[ch], cws[ch]
                s = slice(offs[ch], offs[ch + 1])
                blk = slice(o, o + 3 * cw)
                nc.vector.tensor_tensor(sq[:, blk], xyz_t[:, blk],
                                        xyz_t[:, blk], OP.mult)
                adder = nc.gpsimd if add_eng[ch] == "pool" else nc.vector
                adder.tensor_tensor(
                    t1[:, s], sq[:, o : o + cw], sq[:, o + cw : o + 2 * cw],
                    OP.add)
                adder.tensor_tensor(
                    r2[:, s], t1[:, s], sq[:, o + 2 * cw : o + 3 * cw], OP.add)
                while li < len(ln_cws) and loffs[li + 1] <= offs[ch + 1]:
                    ls = slice(loffs[li], loffs[li + 1])
                    nc.scalar.activation(lnr2[:, ls], r2[:, ls], AF.Ln)
                    nc.scalar.activation(
                        v[:, ls], lnr2[:, ls], AF.Exp,
                        bias=cst_t[:, 0:1], scale=-0.25,
                    )
                    nc.sync.dma_start(out[:, ls], v[:, ls])
                    li += 1

    nc.compile()
    _BASS_CACHE[key] = nc
    _BASS_CACHE["last"] = nc
    return nc


def _build_full():
    """Fallback: full 128-term quadrature kernel (baseline, unchanged)."""
    if "full" in _BASS_CACHE:
        return _BASS_CACHE["full"]
    import concourse.mybir as mybir
    from concourse import bacc
    from concourse.tile import TileContext

    fp32 = mybir.dt.float32
    fp16 = mybir.dt.float16
    AF = mybir.ActivationFunctionType
    OP = mybir.AluOpType

    nc = bacc.Bacc("TRN2")
    xs = nc.dram_tensor("xs", [P, FN], fp32, kind="ExternalInput")
    ys = nc.dram_tensor("ys", [P, FN], fp32, kind="ExternalInput")
    zs = nc.dram_tensor("zs", [P, FN], fp32, kind="ExternalInput")
    w_in = nc.dram_tensor("w_red", [P, G_GRP], fp16, kind="ExternalInput")
    sc_in = nc.dram_tensor("scale_sb", [P, NI], fp32, kind="ExternalInput")
    bi_in = nc.dram_tensor("bias_sb", [P, NI], fp32, kind="ExternalInput")
    ep_in = nc.dram_tensor("eplg", [P, 4], fp32, kind="ExternalInput")
    out = nc.dram_tensor("out", [P, FN], fp32, kind="ExternalOutput")

    with TileContext(nc) as tc:
        with (
            tc.tile_pool(name="singles", bufs=1) as singles,
            tc.tile_pool(name="epool", bufs=4) as epool,
            tc.tile_pool(name="psum", bufs=1, space="PSUM") as psum,
        ):
            x_t = singles.tile([P, FN], fp32)
            y_t = singles.tile([P, FN], fp32)
            z_t = singles.tile([P, FN], fp32)
            w_t = singles.tile([P, G_GRP], fp16)
            sc_t = singles.tile([P, NI], fp32)
            bi_t = singles.tile([P, NI], fp32)
            ep_t = singles.tile([P, 4], fp32)
            nc.sync.dma_start(x_t[:], xs[:])
            nc.sync.dma_start(y_t[:], ys[:])
            nc.sync.dma_start(z_t[:], zs[:])
            nc.sync.dma_start(w_t[:], w_in[:])
            nc.sync.dma_start(sc_t[:], sc_in[:])
            nc.sync.dma_start(bi_t[:], bi_in[:])
            nc.sync.dma_start(ep_t[:], ep_in[:])

            r2 = singles.tile([P, FN], fp32)
            t2 = singles.tile([P, FN], fp32)
            sx = singles.tile([P, FN], fp32)
            nc.scalar.activation(sx[:], x_t[:], AF.Square)
            nc.vector.tensor_tensor(t2[:], y_t[:], y_t[:], OP.mult)
            nc.vector.tensor_tensor(r2[:], z_t[:], z_t[:], OP.mult)
            nc.vector.tensor_tensor(t2[:], t2[:], sx[:], OP.add)
            nc.vector.tensor_tensor(r2[:], r2[:], t2[:], OP.add)

            r2d = singles.tile([P, F], fp32)
            for j in range(D):
                for c in range(D):
                    nc.sync.dma_start(
                        r2d[G_GRP * j : G_GRP * (j + 1), FN * c : FN * (c + 1)],
                        r2[G_GRP * c : G_GRP * (c + 1), :],
                    )

            lnr2n = singles.tile([P, FN], fp32)
            nc.scalar.activation(lnr2n[:], r2[:], AF.Ln)
            bh_n = singles.tile([P, FN], fp32)
            nc.scalar.activation(
                bh_n[:], lnr2n[:], AF.Exp, bias=ep_t[:, 0:1], scale=-1.5
            )

            integ = psum.tile([G_GRP, F], fp32)
            for i in range(NI):
                e = epool.tile([P, F], fp16, tag="e")
                nch = D if i in (0, NI - 1) else 1
                cw = F // nch
                for ch in range(nch):
                    nc.scalar.activation(
                        e[:, cw * ch : cw * (ch + 1)],
                        r2d[:, cw * ch : cw * (ch + 1)],
                        AF.Exp,
                        bias=bi_t[:, i : i + 1], scale=sc_t[:, i : i + 1],
                    )
                for b in range(F // 512):
                    nc.tensor.matmul(
                        integ[:, 512 * b : 512 * (b + 1)],
                        w_t[:],
                        e[:, 512 * b : 512 * (b + 1)],
                        start=(i == 0),
                        stop=(i == NI - 1),
                    )

            mge_g = singles.tile([G_GRP, F], fp32)
            integ_n = singles.tile([P, FN], fp32)
            for c in range(D):
                nc.any.tensor_copy(
                    mge_g[:, FN * c : FN * (c + 1)],
                    integ[:, FN * c : FN * (c + 1)],
                )
                nc.sync.dma_start(
                    integ_n[G_GRP * c : G_GRP * (c + 1), :],
                    mge_g[:, FN * c : FN * (c + 1)],
                )
            vc2 = singles.tile([P, FN], fp32)
            tv = singles.tile([P, FN], fp32)
            lntv = singles.tile([P, FN], fp32)
            v = singles.tile([P, FN], fp32)
            HF = FN // 2
            for h in range(2):
                s = slice(HF * h, HF * (h + 1))
                nc.vector.tensor_tensor(vc2[:, s], integ_n[:, s], bh_n[:, s], OP.add)
                nc.vector.tensor_tensor(tv[:, s], vc2[:, s], r2[:, s], OP.mult)
                nc.scalar.activation(lntv[:, s], tv[:, s], AF.Ln)
                nc.scalar.activation(
                    v[:, s], lntv[:, s], AF.Exp, bias=ep_t[:, 2:3], scale=0.5
                )
                nc.sync.dma_start(out[:, s], v[:, s])

    nc.compile()
    _BASS_CACHE["full"] = nc
    _BASS_CACHE["last"] = nc
    return nc


def _quad_terms(surf, sigma, qobs, M_to_L, inc, quad):
    """fp64 (b_m, c_m) exp-sum terms of vc2_mge in UNSCALED r2, with the
    2*pi*G*scale^2 prefactor folded into c. Mirrors reference.py's math."""
    surf = surf.astype(np.float64)
    sigma = sigma.astype(np.float64)
    qobs = qobs.astype(np.float64)
    cos_i, sin_i = np.cos(inc), np.sin(inc)
    q_intr = np.sqrt(qobs**2 - cos_i**2) / sin_i
    md = surf * M_to_L * qobs / (q_intr * sigma * np.sqrt(2.0 * np.pi))
    scale = np.quantile(sigma, 0.5)
    sig_sc = sigma / scale
    mds = np.quantile(sig_sc, 0.5)
    mxs = sig_sc.max()
    t_lo = np.arcsinh(np.log(1e-7 * mds) * 2.0 / np.pi)
    t_hi = np.arcsinh(np.log(1000.0 * mxs) * 2.0 / np.pi)
    xl, wl = leggauss(quad)
    t = 0.5 * (t_hi - t_lo) * xl + 0.5 * (t_hi + t_lo)
    w = 0.5 * (t_hi - t_lo) * wl
    u = np.exp(np.pi / 2.0 * np.sinh(t))
    du = np.pi / 2.0 * np.cosh(t) * u
    coef = q_intr * md
    inv_s2 = 1.0 / sig_sc**2
    b = ((0.5 / (1.0 + u))[:, None] * inv_s2[None, :]).ravel() / scale**2
    c = (
        (coef[None, :] / ((1.0 + u[:, None]) ** 2
                          * np.sqrt(q_intr[None, :] ** 2 + u[:, None])))
        * (du * w)[:, None]
    ).ravel()
    mge_c = 2.0 * np.pi * G_CONST * scale**2
    return b, c * mge_c, scale


def _x_max(surf, sigma, qobs, M_to_L, inc, m_bh, r2_min, r2_max):
    """max over the data's r2 range of vc2_mge/vc2_bh (exact Q=64 sum)."""
    b, c, scale = _quad_terms(surf, sigma, qobs, M_to_L, inc, 64)
    lo = max(float(r2_min) * 0.5, 1e-30)
    hi = float(r2_max) * 2.0
    grid = np.geomspace(lo, hi, 512)
    f = np.exp(-np.outer(grid, b)) @ c
    bh_coef = G_CONST * 10.0 ** m_bh * scale**2   # vc2_bh = bh_coef*r2^-1.5
    if not np.isfinite(bh_coef) or bh_coef <= 0.0:
        return np.inf
    bh = bh_coef * grid ** -1.5
    return float(np.max(f / bh))


def _host_coeffs_full(surf, sigma, qobs, M_to_L, inc, m_bh):
    """Host-side reduction for the fallback kernel (as in the baseline)."""
    b, c, scale = _quad_terms(surf, sigma, qobs, M_to_L, inc, QUAD)
    b_eff = b
    mge_c = 2.0 * np.pi * G_CONST * scale**2
    assert np.all(c > 0)
    assert c.max() < 6.0e4, "c_m overflows fp16"
    bh_bias = np.log(G_CONST) + m_bh * np.log(10.0) + 2.0 * np.log(scale)
    v_bias = -np.log(scale)
    return b_eff, c, mge_c, bh_bias, v_bias


def _run_fast(x, y, z, m_bh):
    from concourse.bass_utils import run_bass_kernel_spmd

    # pack [x|y|z] chunk-major per core: per chunk of width cw the packed
    # columns are [x_ch | y_ch | z_ch]
    in_np = np.float16 if IN_FP16 else np.float32
    xf = x.ravel().reshape(N_CORES, P, FN).astype(in_np)
    yf = y.ravel().reshape(N_CORES, P, FN).astype(in_np)
    zf = z.ravel().reshape(N_CORES, P, FN).astype(in_np)
    xyz = np.empty((N_CORES, P, 3 * FN), in_np)
    o = 0
    for cw in CWS:
        s = slice(o, o + cw)
        xyz[:, :, 3 * o : 3 * o + cw] = xf[:, :, s]
        xyz[:, :, 3 * o + cw : 3 * o + 2 * cw] = yf[:, :, s]
        xyz[:, :, 3 * o + 2 * cw : 3 * o + 3 * cw] = zf[:, :, s]
        o += cw
    c_val = np.float32(0.5 * (np.log(G_CONST) + float(m_bh) * np.log(10.0)))
    in_maps = [{"xyzp": xyz[i]} for i in range(N_CORES)]
    nc = _build_fast(c_imm=float(c_val))
    res = run_bass_kernel_spmd(nc, in_maps, core_ids=list(range(N_CORES)))
    outs = [res.results[i]["out"].reshape(-1) for i in range(N_CORES)]
    return np.concatenate(outs).reshape(H, W).astype(np.float32)


def _run_full(x, y, z, surf, sigma, qobs, M_to_L, inc, m_bh):
    from concourse.bass_utils import run_bass_kernel_spmd

    b_eff, c, mge_c, bh_bias, v_bias = _host_coeffs_full(
        np.asarray(surf), np.asarray(sigma), np.asarray(qobs),
        float(M_to_L), float(inc), float(m_bh),
    )
    jj = np.arange(P) // G_GRP
    scale_sb = np.empty((P, NI), np.float32)
    bias_sb = np.empty((P, NI), np.float32)
    for i in range(NI):
        m = D * i + jj
        scale_sb[:, i] = -b_eff[m]
        bias_sb[:, i] = np.log(c[m])
    w_red = np.zeros((P, G_GRP), np.float16)
    w_red[np.arange(P), np.arange(P) % G_GRP] = 1.0
    eplg = np.zeros((P, 4), np.float32)
    eplg[:, 0] = bh_bias
    eplg[:, 1] = mge_c
    eplg[:, 2] = v_bias

    xf = x.ravel().reshape(N_CORES, P, FN)
    yf = y.ravel().reshape(N_CORES, P, FN)
    zf = z.ravel().reshape(N_CORES, P, FN)
    in_maps = [
        {
            "xs": xf[i], "ys": yf[i], "zs": zf[i],
            "w_red": w_red, "scale_sb": scale_sb, "bias_sb": bias_sb,
            "eplg": eplg,
        }
        for i in range(N_CORES)
    ]
    nc = _build_full()
    res = run_bass_kernel_spmd(nc, in_maps, core_ids=list(range(N_CORES)))
    outs = [res.results[i]["out"].reshape(-1) for i in range(N_CORES)]
    return np.concatenate(outs).reshape(H, W).astype(np.float32)


def kernel(x, y, z, surf, sigma, qobs, M_to_L, inc, m_bh, quad_points):
    x = np.asarray(x, dtype=np.float32)
    y = np.asarray(y, dtype=np.float32)
    z = np.asarray(z, dtype=np.float32)

    r2 = (x.astype(np.float64) ** 2 + y.astype(np.float64) ** 2
          + z.astype(np.float64) ** 2)
    r2_min, r2_max = float(r2.min()), float(r2.max())
    try:
        xm = _x_max(np.asarray(surf), np.asarray(sigma), np.asarray(qobs),
                    float(M_to_L), float(inc), float(m_bh), r2_min, r2_max)
    except Exception:
        xm = np.inf

    if xm < X_TAYLOR_MAX and r2_min > 0.0:
        return _run_fast(x, y, z, float(m_bh))
    return _run_full(x, y, z, surf, sigma, qobs, M_to_L, inc, m_bh)
